# revision 1
# baseline (speedup 1.0000x reference)
"""Trainium2 Bass kernel for CrossFormerAttention-style GNN message passing.

Reference computation (N=50000 nodes, E=1600000 edges, 8 heads x 16 dims):
    Qh = (h_add @ WQ).reshape(N, 8, 16)
    Kh = (h @ WK).reshape(N, 8, 16)
    Vh = (h @ WV).reshape(N, 8, 16)
    score = sum(Kh[src] * Qh[dst], -1)             # [E, 8, 1]
    wV = segment_sum(Vh[src] * score, dst, N)      # [N, 8, 16]
    out = wV / N

Sharding: edges are partitioned by dst range across the 8 cores (6250 nodes
per core) so each core owns a disjoint slice of the output and no collective
is needed.  Within a core, edges are sorted by dst and grouped into 128-edge
subchunks aligned to 128-node blocks; the segment sum becomes a per-block
one-hot matmul accumulated in PSUM.  K/V rows (fused into one 512B bf16 row
per node) are fetched per edge with the Q7 SWDGE dma_gather.  dma_gather
indices are int16, so each subchunk is split host-side into src<32768 ("A",
table base row 0) and src>=32768 ("B", table base row 32768) subchunks.
"""

import numpy as np

import concourse.bass as bass
import concourse.mybir as mybir
from concourse import bass_utils
from concourse.bacc import Bacc
from concourse.tile import TileContext

P = 128
N_NODES = 50000
N_EDGES = 1600000
IN_DIM = 128
NUM_HEADS = 8
OUT_DIM = 16
N_CORES = 8
NODES_PER_CORE = N_NODES // N_CORES  # 6250
GROUP_J = 8  # subchunks (of 128 edges) per group -> 1024 edges per group
I16_BASE = 32768

F32 = mybir.dt.float32
BF16 = mybir.dt.bfloat16
I16 = mybir.dt.int16


def _ceil_to(x, m):
    return ((x + m - 1) // m) * m


def _to_bf16(a):
    import ml_dtypes

    return np.asarray(a, dtype=np.float32).astype(ml_dtypes.bfloat16)


def shard_edges(src, dst, n_cores=N_CORES, nodes_per_core=NODES_PER_CORE):
    """Partition edges by dst range, sort by dst, split per 128-node block
    into A (src < 32768) and B (src >= 32768) subchunks of 128 edges.
    Schedule (tags / counts) is shared across cores (max over cores) because
    one program runs SPMD on all 8 cores."""
    src = np.asarray(src).astype(np.int64)
    dst = np.asarray(dst).astype(np.int64)

    order = np.argsort(dst, kind="stable")
    ds = dst[order]
    ss = src[order]

    bounds = np.searchsorted(ds, np.arange(n_cores + 1) * nodes_per_core)
    n_blocks = _ceil_to(nodes_per_core, P) // P  # 49

    # per (core, block) A/B edge lists
    edges = [[None] * n_blocks for _ in range(n_cores)]
    nA = np.zeros((n_cores, n_blocks), dtype=np.int64)
    nB = np.zeros((n_cores, n_blocks), dtype=np.int64)
    for c in range(n_cores):
        sl = slice(bounds[c], bounds[c + 1])
        loc = ds[sl] - c * nodes_per_core
        sc = ss[sl]
        blk = loc // P
        bs = np.searchsorted(blk, np.arange(n_blocks + 1))
        for b in range(n_blocks):
            s2 = slice(bs[b], bs[b + 1])
            l2, s3 = loc[s2], sc[s2]
            a_mask = s3 < I16_BASE
            edges[c][b] = (
                (s3[a_mask], l2[a_mask]),
                (s3[~a_mask], l2[~a_mask]),
            )
            nA[c, b] = int(a_mask.sum())
            nB[c, b] = int((~a_mask).sum())

    subA = ((nA.max(axis=0) + P - 1) // P).astype(np.int64)
    subB = ((nB.max(axis=0) + P - 1) // P).astype(np.int64)
    empty = (subA + subB) == 0
    subA[empty] = 1

    tags = []  # per subchunk: 0=A, 1=B
    blk_of = []
    for b in range(n_blocks):
        tags += [0] * int(subA[b]) + [1] * int(subB[b])
        blk_of += [b] * int(subA[b] + subB[b])
    S = len(tags)
    pad = (-S) % GROUP_J
    tags += [0] * pad
    blk_of += [n_blocks - 1] * pad
    S += pad
    G = S // GROUP_J

    # subchunk start offsets per block for A and B regions
    startA = np.zeros(n_blocks, dtype=np.int64)
    startB = np.zeros(n_blocks, dtype=np.int64)
    off = 0
    for b in range(n_blocks):
        startA[b] = off
        startB[b] = off + subA[b]
        off += subA[b] + subB[b]

    kvidx = np.zeros((n_cores, S * P), dtype=np.int16)
    qidx = np.zeros((n_cores, S * P), dtype=np.int16)
    rel = np.full((n_cores, S * P), -1.0, dtype=np.float32)
    for c in range(n_cores):
        for b in range(n_blocks):
            (sa, la), (sb, lb) = edges[c][b]
            o = int(startA[b]) * P
            kvidx[c, o : o + len(sa)] = sa.astype(np.int16)
            qidx[c, o : o + len(sa)] = la.astype(np.int16)
            rel[c, o : o + len(sa)] = la - b * P
            o = int(startB[b]) * P
            kvidx[c, o : o + len(sb)] = (sb - I16_BASE).astype(np.int16)
            qidx[c, o : o + len(sb)] = lb.astype(np.int16)
            rel[c, o : o + len(sb)] = lb - b * P

    # wrapped int16 index layout: edge i of a 512-edge group lives at
    # partition i%16, free i//16; replicated x8 over the 128 partitions.
    def wrap16(a):
        w = a.reshape(n_cores, G, GROUP_J * 8, 16).transpose(0, 1, 3, 2)
        return np.ascontiguousarray(np.tile(w, (1, 1, 8, 1)))

    kvidx_w = wrap16(kvidx)  # [C, G, 128, J*8]
    qidx_w = wrap16(qidx)
    # rel upload [C, G, 128, J]: edge (j, p) -> [g, p, j]
    rel_w = np.ascontiguousarray(
        rel.reshape(n_cores, G, GROUP_J, P).transpose(0, 1, 3, 2)
    )

    return {
        "kvidx_w": kvidx_w,
        "qidx_w": qidx_w,
        "rel_w": rel_w,
        "tags": tags,
        "blk_of": blk_of,
        "n_blocks": n_blocks,
        "S": S,
        "G": G,
    }


def build_program(
    *,
    n_nodes_pad,  # KV table rows (mult of 128)
    nodes_core_pad,  # Q table rows (mult of 128)
    n_groups,
    tags,  # per subchunk 0/1
    blk_of,  # per subchunk block id
    scale,
):
    nkv_chunks = n_nodes_pad // P
    nq_chunks = nodes_core_pad // P
    J = GROUP_J
    S = n_groups * J

    first_of = [i == 0 or blk_of[i] != blk_of[i - 1] for i in range(S)]
    last_of = [i == S - 1 or blk_of[i] != blk_of[i + 1] for i in range(S)]

    # per-group maximal same-tag runs [(j0, j1, tag), ...]
    runs_of_group = []
    for g in range(n_groups):
        runs = []
        j0 = 0
        for j in range(1, J + 1):
            if j == J or tags[g * J + j] != tags[g * J + j0]:
                runs.append((j0, j, tags[g * J + j0]))
                j0 = j
        runs_of_group.append(runs)

    nc = Bacc(num_swdge_queues=4)

    h_pad = nc.dram_tensor("h_pad", [n_nodes_pad, IN_DIM], F32, kind="ExternalInput")
    hq_pad = nc.dram_tensor(
        "hq_pad", [nodes_core_pad, IN_DIM], F32, kind="ExternalInput"
    )
    wq_d = nc.dram_tensor("wq_d", [IN_DIM, IN_DIM], F32, kind="ExternalInput")
    wk_d = nc.dram_tensor("wk_d", [IN_DIM, IN_DIM], F32, kind="ExternalInput")
    wv_d = nc.dram_tensor("wv_d", [IN_DIM, IN_DIM], F32, kind="ExternalInput")
    kvidx_d = nc.dram_tensor(
        "kvidx_d", [n_groups, P, J * 8], I16, kind="ExternalInput"
    )
    qidx_d = nc.dram_tensor("qidx_d", [n_groups, P, J * 8], I16, kind="ExternalInput")
    rel_d = nc.dram_tensor("rel_d", [n_groups, P, J], F32, kind="ExternalInput")
    wv_out = nc.dram_tensor(
        "wv_out", [(max(blk_of) + 1) * P, IN_DIM], F32, kind="ExternalOutput"
    )

    ident_np = np.eye(P, dtype=np.float32)
    ident_d = nc.inline_tensor(ident_np, name="ident_d")
    iota_np = np.tile(np.arange(P, dtype=np.float32), (P, J))  # [P, J*P]
    iota_d = nc.inline_tensor(_to_bf16(iota_np), name="iota_d")

    with TileContext(nc) as tc:
        with (
            tc.tile_pool(name="const", bufs=1) as constp,
            tc.tile_pool(name="dram", bufs=1, space="DRAM") as dramp,
            tc.tile_pool(name="proj", bufs=3) as projp,
            tc.tile_pool(name="proj_ps", bufs=2, space="PSUM") as projps,
            tc.tile_pool(name="gath", bufs=3) as gathp,
            tc.tile_pool(name="work", bufs=3) as workp,
            tc.tile_pool(name="wv_ps", bufs=2, space="PSUM") as wvps,
            tc.tile_pool(name="outst", bufs=3) as outp,
        ):
            ident = constp.tile([P, P], F32)
            nc.sync.dma_start(ident, ident_d[:])
            iota = constp.tile([P, J * P], BF16)
            nc.sync.dma_start(iota, iota_d[:])

            w_sb = {}
            for name, dram in (("wq", wq_d), ("wk", wk_d), ("wv", wv_d)):
                wf = constp.tile([P, IN_DIM], F32, name=f"{name}_f32")
                nc.sync.dma_start(wf, dram[:])
                wb = constp.tile([P, IN_DIM], BF16, name=f"{name}_bf")
                nc.vector.tensor_copy(wb, wf)
                w_sb[name] = wb

            # fused K|V table: one 512B bf16 row per node
            kv_tab = dramp.tile([n_nodes_pad, 2 * IN_DIM], BF16, name="kv_tab")
            q_tab = dramp.tile([nodes_core_pad, IN_DIM], BF16, name="q_tab")

            # ---- Phase A: projections ----
            def project(n_chunks, src_dram, outs):
                # outs: list of (w_tile, dram_table_column_slice)
                for ci in range(n_chunks):
                    rows = src_dram[ci * P : (ci + 1) * P, :]
                    t_h = projp.tile([P, IN_DIM], F32, name="t_h")
                    nc.sync.dma_start(t_h, rows)
                    ps_ht = projps.tile([P, P], F32, name="ps_ht")
                    nc.tensor.transpose(ps_ht, t_h, ident)
                    t_ht = projp.tile([P, P], BF16, name="t_ht")
                    nc.vector.tensor_copy(t_ht, ps_ht)
                    for w_tile, tab, c0 in outs:
                        ps_o = projps.tile([P, IN_DIM], F32, name="ps_o")
                        nc.tensor.matmul(
                            ps_o, lhsT=t_ht, rhs=w_tile, start=True, stop=True
                        )
                        t_o = projp.tile([P, IN_DIM], BF16, name="t_o")
                        nc.scalar.copy(t_o, ps_o)
                        nc.sync.dma_start(
                            tab[ci * P : (ci + 1) * P, c0 : c0 + IN_DIM], t_o
                        )

            project(
                nkv_chunks,
                h_pad,
                [(w_sb["wk"], kv_tab, 0), (w_sb["wv"], kv_tab, IN_DIM)],
            )
            project(nq_chunks, hq_pad, [(w_sb["wq"], q_tab, 0)])

            kv_lo = kv_tab[0:I16_BASE, :]
            kv_hi = kv_tab[I16_BASE:n_nodes_pad, :]

            # ---- Phase B: edge groups ----
            wv_tile = None
            for g in range(n_groups):
                idx_t = gathp.tile([P, J * 8], I16, name="idx_t")
                nc.sync.dma_start(idx_t, kvidx_d[g])
                qidx_t = gathp.tile([P, J * 8], I16, name="qidx_t")
                nc.sync.dma_start(qidx_t, qidx_d[g])
                rel_f = gathp.tile([P, J], F32, name="rel_f")
                nc.sync.dma_start(rel_f, rel_d[g])
                rel_t = gathp.tile([P, J], BF16, name="rel_t")
                nc.vector.tensor_copy(rel_t, rel_f)

                kv_rows = gathp.tile([P, J * 2 * IN_DIM], BF16, name="kv_rows")
                for (j0, j1, tag) in runs_of_group[g]:
                    nidx = (j1 - j0) * P
                    nc.gpsimd.dma_gather(
                        out_ap=kv_rows[
                            :, j0 * 2 * IN_DIM : j1 * 2 * IN_DIM
                        ].rearrange("p (c f) -> p c f", f=2 * IN_DIM),
                        in_ap=kv_hi if tag else kv_lo,
                        idxs_ap=idx_t[:, j0 * 8 : j1 * 8],
                        num_idxs=nidx,
                        num_idxs_reg=nidx,
                        elem_size=2 * IN_DIM,
                        queue_num=g % 2,
                    )
                q_rows = gathp.tile([P, J * IN_DIM], BF16, name="q_rows")
                nc.gpsimd.dma_gather(
                    out_ap=q_rows.rearrange("p (c f) -> p c f", f=IN_DIM),
                    in_ap=q_tab[:],
                    idxs_ap=qidx_t[:],
                    num_idxs=J * P,
                    num_idxs_reg=J * P,
                    elem_size=IN_DIM,
                    queue_num=2 + g % 2,
                )

                onehot = workp.tile([P, J * P], BF16, name="onehot")
                nc.vector.tensor_tensor(
                    out=onehot.rearrange("p (j n) -> p j n", j=J),
                    in0=iota.rearrange("p (j n) -> p j n", j=J),
                    in1=rel_t.unsqueeze(-1).to_broadcast([P, J, P]),
                    op=mybir.AluOpType.is_equal,
                )

                kv3 = kv_rows.rearrange("p (j f) -> p j f", f=2 * IN_DIM)
                k3 = kv3[:, :, 0:IN_DIM]
                v3 = kv3[:, :, IN_DIM : 2 * IN_DIM]

                kq = workp.tile([P, J * IN_DIM], BF16, name="kq")
                nc.vector.tensor_tensor(
                    out=kq.rearrange("p (j f) -> p j f", f=IN_DIM),
                    in0=k3,
                    in1=q_rows.rearrange("p (j f) -> p j f", f=IN_DIM),
                    op=mybir.AluOpType.mult,
                )
                score = workp.tile([P, J * NUM_HEADS], F32, name="score")
                nc.vector.tensor_reduce(
                    out=score,
                    in_=kq.rearrange("p (jh d) -> p jh d", d=OUT_DIM),
                    axis=mybir.AxisListType.X,
                    op=mybir.AluOpType.add,
                )
                score_bf = workp.tile([P, J * NUM_HEADS], BF16, name="score_bf")
                nc.scalar.copy(score_bf, score)
                score_rep = workp.tile([P, J * IN_DIM], BF16, name="score_rep")
                nc.vector.tensor_copy(
                    score_rep.rearrange("p (jh d) -> p jh d", d=OUT_DIM),
                    score_bf.unsqueeze(-1).to_broadcast(
                        [P, J * NUM_HEADS, OUT_DIM]
                    ),
                )
                msg = workp.tile([P, J * IN_DIM], BF16, name="msg")
                nc.vector.tensor_tensor(
                    out=msg.rearrange("p (j f) -> p j f", f=IN_DIM),
                    in0=v3,
                    in1=score_rep.rearrange("p (j f) -> p j f", f=IN_DIM),
                    op=mybir.AluOpType.mult,
                )

                for j in range(J):
                    sc = g * J + j
                    b = blk_of[sc]
                    if first_of[sc]:
                        wv_tile = wvps.tile([P, IN_DIM], F32, name="wv_tile")
                    nc.tensor.matmul(
                        wv_tile,
                        lhsT=onehot[:, j * P : (j + 1) * P],
                        rhs=msg[:, j * IN_DIM : (j + 1) * IN_DIM],
                        start=first_of[sc],
                        stop=last_of[sc],
                    )
                    if last_of[sc]:
                        stage = outp.tile([P, IN_DIM], F32, name="stage")
                        nc.scalar.mul(stage, wv_tile, scale)
                        nc.sync.dma_start(wv_out[b * P : (b + 1) * P, :], stage)

    nc.finalize()
    return nc


def _make_in_maps(h, h_add, WQ, WK, WV, shard, n_nodes_pad, nodes_core_pad):
    h = np.asarray(h, dtype=np.float32)
    h_add = np.asarray(h_add, dtype=np.float32)
    h_p = np.zeros((n_nodes_pad, IN_DIM), dtype=np.float32)
    h_p[:N_NODES] = h
    in_maps = []
    for c in range(N_CORES):
        hq_p = np.zeros((nodes_core_pad, IN_DIM), dtype=np.float32)
        hq_p[:NODES_PER_CORE] = h_add[
            c * NODES_PER_CORE : (c + 1) * NODES_PER_CORE
        ]
        in_maps.append(
            {
                "h_pad": h_p,
                "hq_pad": hq_p,
                "wq_d": np.asarray(WQ, dtype=np.float32),
                "wk_d": np.asarray(WK, dtype=np.float32),
                "wv_d": np.asarray(WV, dtype=np.float32),
                "kvidx_d": shard["kvidx_w"][c],
                "qidx_d": shard["qidx_w"][c],
                "rel_d": shard["rel_w"][c],
            }
        )
    return in_maps


_TRACE = {"trace": False, "last": None, "tmpdir": None}


def kernel(h, h_add, src, dst, WQ, WK, WV):
    shard = shard_edges(src, dst)
    n_nodes_pad = _ceil_to(N_NODES, P)
    nodes_core_pad = _ceil_to(NODES_PER_CORE, P)

    nc = build_program(
        n_nodes_pad=n_nodes_pad,
        nodes_core_pad=nodes_core_pad,
        n_groups=shard["G"],
        tags=shard["tags"],
        blk_of=shard["blk_of"],
        scale=1.0 / N_NODES,
    )
    in_maps = _make_in_maps(h, h_add, WQ, WK, WV, shard, n_nodes_pad, nodes_core_pad)

    res = bass_utils.run_bass_kernel_spmd(
        nc,
        in_maps,
        core_ids=list(range(N_CORES)),
        trace=_TRACE["trace"],
        tmpdir=_TRACE["tmpdir"],
    )
    _TRACE["last"] = res

    out = np.concatenate(
        [np.asarray(res.results[c]["wv_out"])[:NODES_PER_CORE] for c in range(N_CORES)],
        axis=0,
    )
    return out.reshape(N_NODES, NUM_HEADS, OUT_DIM).astype(np.float32)



# revision 18
# speedup vs baseline: 1.7271x; 1.7271x over previous
"""Trainium2 Bass kernel for CrossFormerAttention-style GNN message passing.

Reference (N=50000 nodes, E=1600000 edges, 8 heads x 16 dims):
    Qh = (h_add @ WQ).reshape(N, 8, 16)
    Kh = (h @ WK).reshape(N, 8, 16)
    Vh = (h @ WV).reshape(N, 8, 16)
    score = sum(Kh[src] * Qh[dst], -1)             # [E, 8, 1]
    wV = segment_sum(Vh[src] * score, dst, N)      # [N, 8, 16]
    out = wV / N

Sharding: edges partitioned by dst range across 8 cores (6250 nodes/core);
each core owns a disjoint output slice -> no collective.

v2 design vs baseline:
- Phase A: host uploads h TRANSPOSED in bf16 ([128, n_pad]); per 128-chunk
  the chunk is the PE lhsT directly (no PE transpose, no DVE cast), rhs is
  the fused WK|WV [128, 256]; PSUM->SBUF copies alternate ACT/DVE; stores
  batched 8 chunks per DMA.  Q table stays SBUF-resident (1/N folded into
  WQ on host).
- Phase B: per-edge Q gather replaced by a one-hot matmul against the SBUF
  Q-block (onehotT uploaded per subchunk from host as bf16 - no Q7 cost).
  KV gathers batched per 4096-edge window, 2 calls (A: src<32768 subchunks
  first, B rest) to amortize the ~1us SWDGE fixed cost.  Score replication
  runs on the Scalar engine; all other elementwise work on DVE.
"""

from contextlib import ExitStack

import numpy as np
import ml_dtypes

import concourse.bass as bass
import concourse.mybir as mybir
from concourse import bass_utils
from concourse.bacc import Bacc
from concourse.tile import TileContext

P = 128
N_NODES = 50000
N_EDGES = 1600000
IN_DIM = 128
NUM_HEADS = 8
OUT_DIM = 16
N_CORES = 8
NODES_PER_CORE = N_NODES // N_CORES  # 6250
I16_BASE = 32768
GROUP_J = 8          # subchunks per compute group (1024 edges)
WIN_G = 4            # groups per gather window (4096 edges)
WIN_J = GROUP_J * WIN_G

F32 = mybir.dt.float32
BF16 = mybir.dt.bfloat16
I16 = mybir.dt.int16

ACT_REP = False      # replicate score on Scalar engine (else DVE bcast mult)
GCAP = 8             # max subchunks per gather call (8 -> 1024 idxs)
QSEL_SPLIT = True    # per-subchunk bank-aligned Qsel PSUM tiles


def _ceil_to(x, m):
    return ((x + m - 1) // m) * m


def _bf(a):
    return np.asarray(a, dtype=np.float32).astype(ml_dtypes.bfloat16)


def _bf_bits(a):
    """bf16 bit pattern as int16 (for fused i16 uploads, bitcast on device)."""
    return _bf(a).view(np.int16)


def shard_edges(src, dst):
    """Partition edges by dst range; per core sort by dst; per 128-dst-block
    split into A (src<32768) / B subchunks of 128 edges; shared schedule
    (max counts over cores); subchunks regrouped per 32-subchunk window with
    A-subchunks first so each window needs only 2 gather calls."""
    src = np.asarray(src).astype(np.int64)
    dst = np.asarray(dst).astype(np.int64)

    order = np.argsort(dst, kind="stable")
    ds, ss = dst[order], src[order]
    bounds = np.searchsorted(ds, np.arange(N_CORES + 1) * NODES_PER_CORE)
    n_blocks = _ceil_to(NODES_PER_CORE, P) // P  # 49

    # per (core, block) A/B edge lists
    edges = [[None] * n_blocks for _ in range(N_CORES)]
    nA = np.zeros((N_CORES, n_blocks), dtype=np.int64)
    nB = np.zeros((N_CORES, n_blocks), dtype=np.int64)
    for c in range(N_CORES):
        sl = slice(bounds[c], bounds[c + 1])
        loc = ds[sl] - c * NODES_PER_CORE
        sc = ss[sl]
        bs = np.searchsorted(loc // P, np.arange(n_blocks + 1))
        for b in range(n_blocks):
            s2 = slice(bs[b], bs[b + 1])
            l2, s3 = loc[s2], sc[s2]
            am = s3 < I16_BASE
            edges[c][b] = ((s3[am], l2[am]), (s3[~am], l2[~am]))
            nA[c, b] = int(am.sum())
            nB[c, b] = int((~am).sum())

    subA = ((nA.max(axis=0) + P - 1) // P).astype(np.int64)
    subB = ((nB.max(axis=0) + P - 1) // P).astype(np.int64)
    if (subA + subB).sum() == 0:
        subA[0] = 1

    # subchunk list in block order: (block, tag)
    subs = []
    for b in range(n_blocks):
        subs += [(b, 0)] * int(subA[b]) + [(b, 1)] * int(subB[b])
    pad = (-len(subs)) % WIN_J
    subs += [(n_blocks - 1, 0)] * pad          # dummy subchunks (rel=-1)
    S = len(subs)
    n_win = S // WIN_J

    # reorder within each window: A first, then B (stable keeps block order)
    perm = []
    for w in range(n_win):
        ws = list(range(w * WIN_J, (w + 1) * WIN_J))
        perm += [j for j in ws if subs[j][1] == 0] + [j for j in ws if subs[j][1] == 1]
    subs = [subs[j] for j in perm]
    blk_of = [s[0] for s in subs]
    tags = [s[1] for s in subs]
    nA_win = [sum(1 for j in range(w * WIN_J, (w + 1) * WIN_J) if tags[j] == 0)
              for w in range(n_win)]

    # real (non-dummy) subchunk positions per (block, tag), in order
    slot_of = {}       # (block, tag) -> list of subchunk indices
    for i, (b, t) in enumerate(subs):
        slot_of.setdefault((b, t), []).append(i)
    n_dummy_tail = pad

    # fill per-core data
    kvidx = np.zeros((N_CORES, S * P), dtype=np.int16)
    rel = np.full((N_CORES, S, P), -1.0, dtype=np.float32)
    for c in range(N_CORES):
        for b in range(n_blocks):
            for t in range(2):
                sa, la = edges[c][b][t]
                slots = slot_of.get((b, t), [])
                base = 0 if t == 0 else I16_BASE
                for k, sidx in enumerate(slots):
                    lo = k * P
                    hi = min(lo + P, len(sa))
                    if hi <= lo:
                        break
                    n = hi - lo
                    kvidx[c, sidx * P : sidx * P + n] = (sa[lo:hi] - base).astype(np.int16)
                    rel[c, sidx, :n] = la[lo:hi] - b * P

    # first/last OCCURRENCE per block (A/B reorder interleaves blocks, so a
    # block's subchunks are not contiguous; PSUM tiles keyed by block)
    first_seen, last_seen = {}, {}
    for i, b in enumerate(blk_of):
        if b not in first_seen:
            first_seen[b] = i
        last_seen[b] = i
    first_of = [first_seen[blk_of[i]] == i for i in range(S)]
    last_of = [last_seen[blk_of[i]] == i for i in range(S)]

    # gather idx layout: per window [128, WIN_J*8] int16, wrapped in 16
    # partitions (idx i at partition i%16, col i//16) replicated x8
    kvidx_w = kvidx.reshape(N_CORES, n_win, WIN_J * 8, 16).transpose(0, 1, 3, 2)
    kvidx_w = np.ascontiguousarray(np.tile(kvidx_w, (1, 1, 8, 1)))  # [C,W,128,WIN_J*8]

    # rel per subchunk in partition-edge layout [128, S] (edge on partition)
    rel_pe = rel.transpose(0, 2, 1)                       # [C, P, S]
    rel_bits = _bf_bits(rel_pe)                           # int16 view of bf16

    # fused per-window i16 upload: [128, WIN_J*8 idx | WIN_J rel]
    fused = np.zeros((N_CORES, n_win, P, WIN_J * 8 + WIN_J), dtype=np.int16)
    fused[:, :, :, : WIN_J * 8] = kvidx_w
    fused[:, :, :, WIN_J * 8 :] = rel_bits.reshape(N_CORES, P, n_win, WIN_J).transpose(0, 2, 1, 3)

    # onehotT upload: [S, 128(node), 128(edge)] bf16; rel=-1 -> zero column
    onehotT = np.zeros((N_CORES, S, P, P), dtype=ml_dtypes.bfloat16)
    r = rel.astype(np.int64)                               # [C, S, P]
    cc, ssi, ee = np.nonzero(r >= 0)
    onehotT[cc, ssi, r[cc, ssi, ee], ee] = 1.0

    return {
        "fused": fused,
        "onehotT": onehotT,
        "tags": tags,
        "blk_of": blk_of,
        "first_of": first_of,
        "last_of": last_of,
        "nA_win": nA_win,
        "n_blocks": n_blocks,
        "S": S,
        "n_win": n_win,
    }


def build_program(*, n_nodes_pad, nodes_core_pad, sched):
    nkv_chunks = n_nodes_pad // P          # 391
    nq_chunks = nodes_core_pad // P        # 49
    n_win = sched["n_win"]
    blk_of = sched["blk_of"]
    first_of = sched["first_of"]
    last_of = sched["last_of"]
    nA_win = sched["nA_win"]
    J, W = GROUP_J, WIN_J

    nc = Bacc(num_swdge_queues=4)

    hT_d = nc.dram_tensor("hT_d", [P, n_nodes_pad], BF16, kind="ExternalInput")
    hqT_d = nc.dram_tensor("hqT_d", [P, nodes_core_pad], BF16, kind="ExternalInput")
    wkv_d = nc.dram_tensor("wkv_d", [P, 2 * IN_DIM], BF16, kind="ExternalInput")
    wq_d = nc.dram_tensor("wq_d", [P, IN_DIM], BF16, kind="ExternalInput")
    fused_d = nc.dram_tensor("fused_d", [n_win, P, W * 8 + W], I16, kind="ExternalInput")
    ohT_d = nc.dram_tensor("ohT_d", [sched["S"], P, P], BF16, kind="ExternalInput")
    wv_out = nc.dram_tensor(
        "wv_out", [sched["n_blocks"] * P, IN_DIM], F32, kind="ExternalOutput"
    )

    iota_np = np.tile(np.arange(P, dtype=np.float32), (P, J))  # [P, J*P]
    iota_d = nc.inline_tensor(_bf(iota_np), name="iota_d")

    with TileContext(nc) as tc:
        with (
            tc.tile_pool(name="const", bufs=1) as constp,
            tc.tile_pool(name="dram", bufs=1, space="DRAM") as dramp,
        ):
            iota = constp.tile([P, J * P], BF16)
            nc.sync.dma_start(iota, iota_d[:])
            wkv = constp.tile([P, 2 * IN_DIM], BF16, name="wkv")
            nc.sync.dma_start(wkv, wkv_d[:])
            wq = constp.tile([P, IN_DIM], BF16, name="wq")
            nc.sync.dma_start(wq, wq_d[:])

            kv_tab = dramp.tile([n_nodes_pad, 2 * IN_DIM], BF16, name="kv_tab")
            q_tab = constp.tile([P, nq_chunks * IN_DIM], BF16, name="q_tab")

            # ---- Phase A: projections (pools freed before phase B) ----
            SLAB = 16  # chunks per hT slab load (16*128 cols = 0.5MB)
            with (
                tc.tile_pool(name="hslab", bufs=2) as hslabp,
                tc.tile_pool(name="proj_ps", bufs=2, space="PSUM") as projps,
                tc.tile_pool(name="kvstage", bufs=2) as kvstagep,
            ):
                for s0 in range(0, nkv_chunks, SLAB):
                    s1 = min(s0 + SLAB, nkv_chunks)
                    slab = hslabp.tile([P, SLAB * P], BF16, name="slab")
                    nc.sync.dma_start(slab[:, : (s1 - s0) * P], hT_d[:, s0 * P : s1 * P])
                    stage = kvstagep.tile([P, SLAB * 2 * IN_DIM], BF16, name="kvstage")
                    for ci in range(s0, s1):
                        k = ci - s0
                        ps = projps.tile([P, 2 * IN_DIM], F32, name="ps_kv")
                        nc.tensor.matmul(
                            ps, lhsT=slab[:, k * P : (k + 1) * P], rhs=wkv,
                            start=True, stop=True,
                        )
                        dst = stage[:, k * 2 * IN_DIM : (k + 1) * 2 * IN_DIM]
                        if ci % 2 == 0:
                            nc.scalar.copy(dst, ps)
                        else:
                            nc.vector.tensor_copy(dst, ps)
                    nc.sync.dma_start(
                        kv_tab[s0 * P : s1 * P, :].rearrange(
                            "(c p) f -> p c f", p=P
                        ),
                        stage.rearrange("p (c f) -> p c f", f=2 * IN_DIM)[:, : s1 - s0, :],
                    )
                # Q projections -> SBUF-resident q_tab
                hq_slab = hslabp.tile([P, nq_chunks * P], BF16, name="hq_slab")
                nc.sync.dma_start(hq_slab, hqT_d[:, : nq_chunks * P])
                for ci in range(nq_chunks):
                    ps = projps.tile([P, IN_DIM], F32, name="ps_q")
                    nc.tensor.matmul(
                        ps, lhsT=hq_slab[:, ci * P : (ci + 1) * P], rhs=wq,
                        start=True, stop=True,
                    )
                    if ci % 2 == 0:
                        nc.scalar.copy(q_tab[:, ci * IN_DIM : (ci + 1) * IN_DIM], ps)
                    else:
                        nc.vector.tensor_copy(q_tab[:, ci * IN_DIM : (ci + 1) * IN_DIM], ps)

            kv_lo = kv_tab[0:I16_BASE, :]
            kv_hi = kv_tab[I16_BASE:n_nodes_pad, :]

            # ---- Phase B ----
            stack = ExitStack()
            winp = stack.enter_context(tc.tile_pool(name="win", bufs=2))
            grpp = stack.enter_context(tc.tile_pool(name="grp", bufs=2))
            qselps = stack.enter_context(
                tc.tile_pool(name="qsel_ps", bufs=2, space="PSUM")
            )
            wvps = stack.enter_context(tc.tile_pool(name="wv_ps", bufs=3, space="PSUM"))
            outp = stack.enter_context(tc.tile_pool(name="outst", bufs=3))
            wv_tiles = {}
            for w in range(n_win):
                fused_t = winp.tile([P, W * 8 + W], I16, name="fused_t")
                nc.sync.dma_start(fused_t, fused_d[w])
                kv_rows = winp.tile([P, W * 2 * IN_DIM], BF16, name="kv_rows")
                nA = nA_win[w]
                runs = []
                if nA > 0:
                    runs.append((0, nA, 0))
                if nA < W:
                    runs.append((nA, W, 1))
                split_runs = []
                for (j0, j1, tag) in runs:
                    for jk in range(j0, j1, GCAP):
                        split_runs.append((jk, min(jk + GCAP, j1), tag))
                for (j0, j1, tag) in split_runs:
                    nidx = (j1 - j0) * P
                    nc.gpsimd.dma_gather(
                        out_ap=kv_rows[
                            :, j0 * 2 * IN_DIM : j1 * 2 * IN_DIM
                        ].rearrange("p (c f) -> p c f", f=2 * IN_DIM),
                        in_ap=kv_hi if tag else kv_lo,
                        idxs_ap=fused_t[:, j0 * 8 : j1 * 8],
                        num_idxs=nidx,
                        num_idxs_reg=nidx,
                        elem_size=2 * IN_DIM,
                        queue_num=(2 * w + tag) % 4,
                    )
                rel_all = fused_t[:, W * 8 :].bitcast(BF16)  # [P, W] bf16

                for g in range(WIN_G):
                    sc0 = w * W + g * J    # first subchunk index of group
                    kvg = kv_rows[:, g * J * 2 * IN_DIM : (g + 1) * J * 2 * IN_DIM]
                    kv3 = kvg.rearrange("p (j f) -> p j f", f=2 * IN_DIM)
                    k3 = kv3[:, :, 0:IN_DIM]
                    v3 = kv3[:, :, IN_DIM : 2 * IN_DIM]

                    # scatter one-hot [edge, node] via iota == rel
                    onehot = grpp.tile([P, J * P], BF16, name="onehot")
                    nc.vector.tensor_tensor(
                        out=onehot.rearrange("p (j n) -> p j n", j=J),
                        in0=iota.rearrange("p (j n) -> p j n", j=J),
                        in1=rel_all[:, g * J : (g + 1) * J]
                        .unsqueeze(-1)
                        .to_broadcast([P, J, P]),
                        op=mybir.AluOpType.is_equal,
                    )
                    # Q selection one-hot (transposed) from host
                    ohT = grpp.tile([P, J * P], BF16, name="ohT")
                    nc.sync.dma_start(
                        ohT.rearrange("p (j n) -> p j n", j=J),
                        ohT_d[sc0 : sc0 + J].rearrange("j p n -> p j n"),
                    )
                    # Qsel[e, f] per subchunk via PE, then kq = K * Qsel
                    kq = grpp.tile([P, J * IN_DIM], BF16, name="kq")
                    if QSEL_SPLIT:
                        for j in range(J):
                            b = blk_of[sc0 + j]
                            qsel_ps = qselps.tile([P, IN_DIM], F32, name="qsel_ps")
                            nc.tensor.matmul(
                                qsel_ps,
                                lhsT=ohT[:, j * P : (j + 1) * P],
                                rhs=q_tab[:, b * IN_DIM : (b + 1) * IN_DIM],
                                start=True, stop=True,
                            )
                            nc.vector.tensor_tensor(
                                out=kq[:, j * IN_DIM : (j + 1) * IN_DIM],
                                in0=k3[:, j, :],
                                in1=qsel_ps,
                                op=mybir.AluOpType.mult,
                            )
                    else:
                        qsel_ps = qselps.tile([P, J * IN_DIM], F32, name="qsel_ps")
                        for j in range(J):
                            b = blk_of[sc0 + j]
                            nc.tensor.matmul(
                                qsel_ps[:, j * IN_DIM : (j + 1) * IN_DIM],
                                lhsT=ohT[:, j * P : (j + 1) * P],
                                rhs=q_tab[:, b * IN_DIM : (b + 1) * IN_DIM],
                                start=True, stop=True,
                            )
                        nc.vector.tensor_tensor(
                            out=kq.rearrange("p (j f) -> p j f", f=IN_DIM),
                            in0=k3,
                            in1=qsel_ps.rearrange("p (j f) -> p j f", f=IN_DIM),
                            op=mybir.AluOpType.mult,
                        )
                    # score[e, (j,h)] = reduce_d kq
                    score = grpp.tile([P, J * NUM_HEADS], F32, name="score")
                    nc.vector.tensor_reduce(
                        out=score,
                        in_=kq.rearrange("p (jh d) -> p jh d", d=OUT_DIM),
                        axis=mybir.AxisListType.X,
                        op=mybir.AluOpType.add,
                    )
                    msg = grpp.tile([P, J * IN_DIM], BF16, name="msg")
                    if ACT_REP:
                        score_rep = grpp.tile([P, J * IN_DIM], BF16, name="score_rep")
                        nc.scalar.copy(
                            score_rep.rearrange("p (jh d) -> p jh d", d=OUT_DIM),
                            score.unsqueeze(-1).to_broadcast(
                                [P, J * NUM_HEADS, OUT_DIM]
                            ),
                        )
                        nc.vector.tensor_tensor(
                            out=msg.rearrange("p (j f) -> p j f", f=IN_DIM),
                            in0=v3,
                            in1=score_rep.rearrange("p (j f) -> p j f", f=IN_DIM),
                            op=mybir.AluOpType.mult,
                        )
                    else:
                        for j in range(J):
                            nc.vector.tensor_tensor(
                                out=msg[:, j * IN_DIM : (j + 1) * IN_DIM].rearrange(
                                    "p (h d) -> p h d", d=OUT_DIM
                                ),
                                in0=v3[:, j, :].rearrange("p (h d) -> p h d", d=OUT_DIM),
                                in1=score[:, j * NUM_HEADS : (j + 1) * NUM_HEADS]
                                .unsqueeze(-1)
                                .to_broadcast([P, NUM_HEADS, OUT_DIM]),
                                op=mybir.AluOpType.mult,
                            )
                    # scatter-accumulate into block tiles
                    for j in range(J):
                        sc = sc0 + j
                        b = blk_of[sc]
                        if first_of[sc]:
                            wv_tiles[b] = wvps.tile([P, IN_DIM], F32, name="wv_tile")
                        nc.tensor.matmul(
                            wv_tiles[b],
                            lhsT=onehot[:, j * P : (j + 1) * P],
                            rhs=msg[:, j * IN_DIM : (j + 1) * IN_DIM],
                            start=first_of[sc],
                            stop=last_of[sc],
                        )
                        if last_of[sc]:
                            stage = outp.tile([P, IN_DIM], F32, name="stage")
                            nc.scalar.copy(stage, wv_tiles.pop(b))
                            nc.sync.dma_start(wv_out[b * P : (b + 1) * P, :], stage)
            stack.close()

    nc.finalize()
    return nc


def _make_in_maps(h, h_add, WQ, WK, WV, sched, n_nodes_pad, nodes_core_pad):
    h = np.asarray(h, dtype=np.float32)
    h_add = np.asarray(h_add, dtype=np.float32)
    hT = np.zeros((P, n_nodes_pad), dtype=ml_dtypes.bfloat16)
    hT[:, :N_NODES] = _bf(h.T)
    wkv = np.concatenate(
        [np.asarray(WK, np.float32), np.asarray(WV, np.float32)], axis=1
    )
    wkv = _bf(wkv)
    wq = _bf(np.asarray(WQ, np.float32) / float(N_NODES))
    in_maps = []
    for c in range(N_CORES):
        hqT = np.zeros((P, nodes_core_pad), dtype=ml_dtypes.bfloat16)
        hqT[:, :NODES_PER_CORE] = _bf(
            h_add[c * NODES_PER_CORE : (c + 1) * NODES_PER_CORE].T
        )
        in_maps.append(
            {
                "hT_d": hT,
                "hqT_d": hqT,
                "wkv_d": wkv,
                "wq_d": wq,
                "fused_d": sched["fused"][c],
                "ohT_d": sched["onehotT"][c],
            }
        )
    return in_maps


_TRACE = {"trace": False, "last": None, "tmpdir": None}


def kernel(h, h_add, src, dst, WQ, WK, WV):
    sched = shard_edges(src, dst)
    n_nodes_pad = _ceil_to(N_NODES, P)
    nodes_core_pad = _ceil_to(NODES_PER_CORE, P)

    nc = build_program(
        n_nodes_pad=n_nodes_pad, nodes_core_pad=nodes_core_pad, sched=sched
    )
    in_maps = _make_in_maps(h, h_add, WQ, WK, WV, sched, n_nodes_pad, nodes_core_pad)

    res = bass_utils.run_bass_kernel_spmd(
        nc,
        in_maps,
        core_ids=list(range(N_CORES)),
        trace=_TRACE["trace"],
        tmpdir=_TRACE["tmpdir"],
    )
    _TRACE["last"] = res

    out = np.concatenate(
        [np.asarray(res.results[c]["wv_out"])[:NODES_PER_CORE] for c in range(N_CORES)],
        axis=0,
    )
    return out.reshape(N_NODES, NUM_HEADS, OUT_DIM).astype(np.float32)


# revision 19
# speedup vs baseline: 2.3749x; 1.3751x over previous
"""Trainium2 Bass kernel for CrossFormerAttention-style GNN message passing.

Reference (N=50000 nodes, E=1600000 edges, 8 heads x 16 dims):
    Qh = (h_add @ WQ).reshape(N, 8, 16)
    Kh = (h @ WK).reshape(N, 8, 16)
    Vh = (h @ WV).reshape(N, 8, 16)
    score = sum(Kh[src] * Qh[dst], -1)             # [E, 8, 1]
    wV = segment_sum(Vh[src] * score, dst, N)      # [N, 8, 16]
    out = wV / N

Sharding: edges partitioned by dst range across 8 cores (6250 nodes/core);
each core owns a disjoint output slice -> no collective.

v2 design vs baseline:
- Phase A: host uploads h TRANSPOSED in bf16 ([128, n_pad]); per 128-chunk
  the chunk is the PE lhsT directly (no PE transpose, no DVE cast), rhs is
  the fused WK|WV [128, 256]; PSUM->SBUF copies alternate ACT/DVE; stores
  batched 8 chunks per DMA.  Q table stays SBUF-resident (1/N folded into
  WQ on host).
- Phase B: per-edge Q gather replaced by a one-hot matmul against the SBUF
  Q-block (onehotT uploaded per subchunk from host as bf16 - no Q7 cost).
  KV gathers batched per 4096-edge window, 2 calls (A: src<32768 subchunks
  first, B rest) to amortize the ~1us SWDGE fixed cost.  Score replication
  runs on the Scalar engine; all other elementwise work on DVE.
"""

from contextlib import ExitStack

import numpy as np
import ml_dtypes

import concourse.bass as bass
import concourse.mybir as mybir
from concourse import bass_utils
from concourse.bacc import Bacc
from concourse.tile import TileContext

P = 128
N_NODES = 50000
N_EDGES = 1600000
IN_DIM = 128
NUM_HEADS = 8
OUT_DIM = 16
N_CORES = 8
NODES_PER_CORE = N_NODES // N_CORES  # 6250
I16_BASE = 32768
GROUP_J = 8          # subchunks per compute group (1024 edges)
WIN_G = 4            # groups per gather window (4096 edges)
WIN_J = GROUP_J * WIN_G

F32 = mybir.dt.float32
BF16 = mybir.dt.bfloat16
I16 = mybir.dt.int16

ACT_REP = True      # replicate score on Scalar engine (else DVE bcast mult)
GCAP = 8             # max subchunks per gather call (8 -> 1024 idxs)
QSEL_SPLIT = False    # per-subchunk bank-aligned Qsel PSUM tiles


def _ceil_to(x, m):
    return ((x + m - 1) // m) * m


def _bf(a):
    return np.asarray(a, dtype=np.float32).astype(ml_dtypes.bfloat16)


def _bf_bits(a):
    """bf16 bit pattern as int16 (for fused i16 uploads, bitcast on device)."""
    return _bf(a).view(np.int16)


def shard_edges(src, dst):
    """Partition edges by dst range; per core sort by dst; per 128-dst-block
    split into A (src<32768) / B subchunks of 128 edges; shared schedule
    (max counts over cores); subchunks regrouped per 32-subchunk window with
    A-subchunks first so each window needs only 2 gather calls."""
    src = np.asarray(src).astype(np.int64)
    dst = np.asarray(dst).astype(np.int64)

    order = np.argsort(dst, kind="stable")
    ds, ss = dst[order], src[order]
    bounds = np.searchsorted(ds, np.arange(N_CORES + 1) * NODES_PER_CORE)
    n_blocks = _ceil_to(NODES_PER_CORE, P) // P  # 49

    # per (core, block) A/B edge lists
    edges = [[None] * n_blocks for _ in range(N_CORES)]
    nA = np.zeros((N_CORES, n_blocks), dtype=np.int64)
    nB = np.zeros((N_CORES, n_blocks), dtype=np.int64)
    for c in range(N_CORES):
        sl = slice(bounds[c], bounds[c + 1])
        loc = ds[sl] - c * NODES_PER_CORE
        sc = ss[sl]
        bs = np.searchsorted(loc // P, np.arange(n_blocks + 1))
        for b in range(n_blocks):
            s2 = slice(bs[b], bs[b + 1])
            l2, s3 = loc[s2], sc[s2]
            am = s3 < I16_BASE
            edges[c][b] = ((s3[am], l2[am]), (s3[~am], l2[~am]))
            nA[c, b] = int(am.sum())
            nB[c, b] = int((~am).sum())

    subA = ((nA.max(axis=0) + P - 1) // P).astype(np.int64)
    subB = ((nB.max(axis=0) + P - 1) // P).astype(np.int64)
    if (subA + subB).sum() == 0:
        subA[0] = 1

    # subchunk list in block order: (block, tag)
    subs = []
    for b in range(n_blocks):
        subs += [(b, 0)] * int(subA[b]) + [(b, 1)] * int(subB[b])
    pad = (-len(subs)) % WIN_J
    subs += [(n_blocks - 1, 0)] * pad          # dummy subchunks (rel=-1)
    S = len(subs)
    n_win = S // WIN_J

    # reorder within each window: A first, then B (stable keeps block order)
    perm = []
    for w in range(n_win):
        ws = list(range(w * WIN_J, (w + 1) * WIN_J))
        perm += [j for j in ws if subs[j][1] == 0] + [j for j in ws if subs[j][1] == 1]
    subs = [subs[j] for j in perm]
    blk_of = [s[0] for s in subs]
    tags = [s[1] for s in subs]
    nA_win = [sum(1 for j in range(w * WIN_J, (w + 1) * WIN_J) if tags[j] == 0)
              for w in range(n_win)]

    # real (non-dummy) subchunk positions per (block, tag), in order
    slot_of = {}       # (block, tag) -> list of subchunk indices
    for i, (b, t) in enumerate(subs):
        slot_of.setdefault((b, t), []).append(i)
    n_dummy_tail = pad

    # fill per-core data
    kvidx = np.zeros((N_CORES, S * P), dtype=np.int16)
    rel = np.full((N_CORES, S, P), -1.0, dtype=np.float32)
    for c in range(N_CORES):
        for b in range(n_blocks):
            for t in range(2):
                sa, la = edges[c][b][t]
                slots = slot_of.get((b, t), [])
                base = 0 if t == 0 else I16_BASE
                for k, sidx in enumerate(slots):
                    lo = k * P
                    hi = min(lo + P, len(sa))
                    if hi <= lo:
                        break
                    n = hi - lo
                    kvidx[c, sidx * P : sidx * P + n] = (sa[lo:hi] - base).astype(np.int16)
                    rel[c, sidx, :n] = la[lo:hi] - b * P

    # first/last OCCURRENCE per block (A/B reorder interleaves blocks, so a
    # block's subchunks are not contiguous; PSUM tiles keyed by block)
    first_seen, last_seen = {}, {}
    for i, b in enumerate(blk_of):
        if b not in first_seen:
            first_seen[b] = i
        last_seen[b] = i
    first_of = [first_seen[blk_of[i]] == i for i in range(S)]
    last_of = [last_seen[blk_of[i]] == i for i in range(S)]

    # gather idx layout: per window [128, WIN_J*8] int16, wrapped in 16
    # partitions (idx i at partition i%16, col i//16) replicated x8
    kvidx_w = kvidx.reshape(N_CORES, n_win, WIN_J * 8, 16).transpose(0, 1, 3, 2)
    kvidx_w = np.ascontiguousarray(np.tile(kvidx_w, (1, 1, 8, 1)))  # [C,W,128,WIN_J*8]

    # rel per subchunk in partition-edge layout [128, S] (edge on partition)
    rel_pe = rel.transpose(0, 2, 1)                       # [C, P, S]
    rel_bits = _bf_bits(rel_pe)                           # int16 view of bf16

    # fused per-window i16 upload: [128, WIN_J*8 idx | WIN_J rel]
    fused = np.zeros((N_CORES, n_win, P, WIN_J * 8 + WIN_J), dtype=np.int16)
    fused[:, :, :, : WIN_J * 8] = kvidx_w
    fused[:, :, :, WIN_J * 8 :] = rel_bits.reshape(N_CORES, P, n_win, WIN_J).transpose(0, 2, 1, 3)

    # onehotT upload, group-contiguous: [n_groups, 128(node), GROUP_J*128]
    # (ohT_g[g, p, j*128+n] = 1 iff rel[subchunk g*J+j, edge n] == p)
    onehotT = np.zeros((N_CORES, S, P, P), dtype=ml_dtypes.bfloat16)
    r = rel.astype(np.int64)                               # [C, S, P]
    cc, ssi, ee = np.nonzero(r >= 0)
    onehotT[cc, ssi, r[cc, ssi, ee], ee] = 1.0
    n_groups = S // GROUP_J
    onehotT = np.ascontiguousarray(
        onehotT.reshape(N_CORES, n_groups, GROUP_J, P, P).transpose(0, 1, 3, 2, 4)
    ).reshape(N_CORES, n_groups, P, GROUP_J * P)

    return {
        "fused": fused,
        "onehotT": onehotT,
        "tags": tags,
        "blk_of": blk_of,
        "first_of": first_of,
        "last_of": last_of,
        "nA_win": nA_win,
        "n_blocks": n_blocks,
        "S": S,
        "n_win": n_win,
    }


def build_program(*, n_nodes_pad, nodes_core_pad, sched):
    nkv_chunks = n_nodes_pad // P          # 391
    nq_chunks = nodes_core_pad // P        # 49
    n_win = sched["n_win"]
    blk_of = sched["blk_of"]
    first_of = sched["first_of"]
    last_of = sched["last_of"]
    nA_win = sched["nA_win"]
    J, W = GROUP_J, WIN_J

    nc = Bacc(num_swdge_queues=4)

    hT_d = nc.dram_tensor("hT_d", [P, n_nodes_pad], BF16, kind="ExternalInput")
    hqT_d = nc.dram_tensor("hqT_d", [P, nodes_core_pad], BF16, kind="ExternalInput")
    wkv_d = nc.dram_tensor("wkv_d", [P, 2 * IN_DIM], BF16, kind="ExternalInput")
    wq_d = nc.dram_tensor("wq_d", [P, IN_DIM], BF16, kind="ExternalInput")
    fused_d = nc.dram_tensor("fused_d", [n_win, P, W * 8 + W], I16, kind="ExternalInput")
    ohT_d = nc.dram_tensor(
        "ohT_d", [sched["S"] // GROUP_J, P, GROUP_J * P], BF16, kind="ExternalInput"
    )
    wv_out = nc.dram_tensor(
        "wv_out", [sched["n_blocks"] * P, IN_DIM], F32, kind="ExternalOutput"
    )

    iota_np = np.tile(np.arange(P, dtype=np.float32), (P, J))  # [P, J*P]
    iota_d = nc.inline_tensor(_bf(iota_np), name="iota_d")

    with TileContext(nc) as tc:
        with (
            tc.tile_pool(name="const", bufs=1) as constp,
            tc.tile_pool(name="dram", bufs=1, space="DRAM") as dramp,
        ):
            iota = constp.tile([P, J * P], BF16)
            nc.sync.dma_start(iota, iota_d[:])
            wkv = constp.tile([P, 2 * IN_DIM], BF16, name="wkv")
            nc.sync.dma_start(wkv, wkv_d[:])
            wq = constp.tile([P, IN_DIM], BF16, name="wq")
            nc.sync.dma_start(wq, wq_d[:])

            kv_tab = dramp.tile([n_nodes_pad, 2 * IN_DIM], BF16, name="kv_tab")
            q_tab = constp.tile([P, nq_chunks * IN_DIM], BF16, name="q_tab")

            # ---- Phase A: projections (pools freed before phase B) ----
            SLAB = 16  # chunks per hT slab load (16*128 cols = 0.5MB)
            with (
                tc.tile_pool(name="hslab", bufs=2) as hslabp,
                tc.tile_pool(name="proj_ps", bufs=2, space="PSUM") as projps,
                tc.tile_pool(name="kvstage", bufs=2) as kvstagep,
            ):
                for s0 in range(0, nkv_chunks, SLAB):
                    s1 = min(s0 + SLAB, nkv_chunks)
                    slab = hslabp.tile([P, SLAB * P], BF16, name="slab")
                    nc.sync.dma_start(slab[:, : (s1 - s0) * P], hT_d[:, s0 * P : s1 * P])
                    stage = kvstagep.tile([P, SLAB * 2 * IN_DIM], BF16, name="kvstage")
                    for ci in range(s0, s1):
                        k = ci - s0
                        ps = projps.tile([P, 2 * IN_DIM], F32, name="ps_kv")
                        nc.tensor.matmul(
                            ps, lhsT=slab[:, k * P : (k + 1) * P], rhs=wkv,
                            start=True, stop=True,
                        )
                        dst = stage[:, k * 2 * IN_DIM : (k + 1) * 2 * IN_DIM]
                        if ci % 2 == 0:
                            nc.scalar.copy(dst, ps)
                        else:
                            nc.vector.tensor_copy(dst, ps)
                    nc.sync.dma_start(
                        kv_tab[s0 * P : s1 * P, :].rearrange(
                            "(c p) f -> p c f", p=P
                        ),
                        stage.rearrange("p (c f) -> p c f", f=2 * IN_DIM)[:, : s1 - s0, :],
                    )
                # Q projections -> SBUF-resident q_tab
                hq_slab = hslabp.tile([P, nq_chunks * P], BF16, name="hq_slab")
                nc.sync.dma_start(hq_slab, hqT_d[:, : nq_chunks * P])
                for ci in range(nq_chunks):
                    ps = projps.tile([P, IN_DIM], F32, name="ps_q")
                    nc.tensor.matmul(
                        ps, lhsT=hq_slab[:, ci * P : (ci + 1) * P], rhs=wq,
                        start=True, stop=True,
                    )
                    if ci % 2 == 0:
                        nc.scalar.copy(q_tab[:, ci * IN_DIM : (ci + 1) * IN_DIM], ps)
                    else:
                        nc.vector.tensor_copy(q_tab[:, ci * IN_DIM : (ci + 1) * IN_DIM], ps)

            kv_lo = kv_tab[0:I16_BASE, :]
            kv_hi = kv_tab[I16_BASE:n_nodes_pad, :]

            # ---- Phase B ----
            gq = [0]  # rotating SWDGE queue counter
            stack = ExitStack()
            winp = stack.enter_context(tc.tile_pool(name="win", bufs=2))
            grpp = stack.enter_context(tc.tile_pool(name="grp", bufs=2))
            qselps = stack.enter_context(
                tc.tile_pool(name="qsel_ps", bufs=2, space="PSUM")
            )
            wvps = stack.enter_context(tc.tile_pool(name="wv_ps", bufs=3, space="PSUM"))
            outp = stack.enter_context(tc.tile_pool(name="outst", bufs=3))
            wv_tiles = {}
            for w in range(n_win):
                fused_t = winp.tile([P, W * 8 + W], I16, name="fused_t")
                nc.sync.dma_start(fused_t, fused_d[w])
                kv_rows = winp.tile([P, W * 2 * IN_DIM], BF16, name="kv_rows")
                nA = nA_win[w]
                runs = []
                if nA > 0:
                    runs.append((0, nA, 0))
                if nA < W:
                    runs.append((nA, W, 1))
                split_runs = []
                for (j0, j1, tag) in runs:
                    for jk in range(j0, j1, GCAP):
                        split_runs.append((jk, min(jk + GCAP, j1), tag))
                for (j0, j1, tag) in split_runs:
                    nidx = (j1 - j0) * P
                    nc.gpsimd.dma_gather(
                        out_ap=kv_rows[
                            :, j0 * 2 * IN_DIM : j1 * 2 * IN_DIM
                        ].rearrange("p (c f) -> p c f", f=2 * IN_DIM),
                        in_ap=kv_hi if tag else kv_lo,
                        idxs_ap=fused_t[:, j0 * 8 : j1 * 8],
                        num_idxs=nidx,
                        num_idxs_reg=nidx,
                        elem_size=2 * IN_DIM,
                        queue_num=gq[0] % 4,
                    )
                    gq[0] += 1
                rel_all = fused_t[:, W * 8 :].bitcast(BF16)  # [P, W] bf16

                for g in range(WIN_G):
                    sc0 = w * W + g * J    # first subchunk index of group
                    kvg = kv_rows[:, g * J * 2 * IN_DIM : (g + 1) * J * 2 * IN_DIM]
                    kv3 = kvg.rearrange("p (j f) -> p j f", f=2 * IN_DIM)
                    k3 = kv3[:, :, 0:IN_DIM]
                    v3 = kv3[:, :, IN_DIM : 2 * IN_DIM]

                    # scatter one-hot [edge, node] via iota == rel
                    onehot = grpp.tile([P, J * P], BF16, name="onehot")
                    nc.vector.tensor_tensor(
                        out=onehot.rearrange("p (j n) -> p j n", j=J),
                        in0=iota.rearrange("p (j n) -> p j n", j=J),
                        in1=rel_all[:, g * J : (g + 1) * J]
                        .unsqueeze(-1)
                        .to_broadcast([P, J, P]),
                        op=mybir.AluOpType.is_equal,
                    )
                    # Q selection one-hot (transposed) from host
                    ohT = grpp.tile([P, J * P], BF16, name="ohT")
                    nc.sync.dma_start(ohT, ohT_d[sc0 // J])
                    # Qsel[e, f] per subchunk via PE, then kq = K * Qsel
                    kq = grpp.tile([P, J * IN_DIM], BF16, name="kq")
                    if QSEL_SPLIT:
                        for j in range(J):
                            b = blk_of[sc0 + j]
                            qsel_ps = qselps.tile([P, IN_DIM], F32, name="qsel_ps")
                            nc.tensor.matmul(
                                qsel_ps,
                                lhsT=ohT[:, j * P : (j + 1) * P],
                                rhs=q_tab[:, b * IN_DIM : (b + 1) * IN_DIM],
                                start=True, stop=True,
                            )
                            nc.vector.tensor_tensor(
                                out=kq[:, j * IN_DIM : (j + 1) * IN_DIM],
                                in0=k3[:, j, :],
                                in1=qsel_ps,
                                op=mybir.AluOpType.mult,
                            )
                    else:
                        qsel_ps = qselps.tile([P, J * IN_DIM], F32, name="qsel_ps")
                        for j in range(J):
                            b = blk_of[sc0 + j]
                            nc.tensor.matmul(
                                qsel_ps[:, j * IN_DIM : (j + 1) * IN_DIM],
                                lhsT=ohT[:, j * P : (j + 1) * P],
                                rhs=q_tab[:, b * IN_DIM : (b + 1) * IN_DIM],
                                start=True, stop=True,
                            )
                        nc.vector.tensor_tensor(
                            out=kq.rearrange("p (j f) -> p j f", f=IN_DIM),
                            in0=k3,
                            in1=qsel_ps.rearrange("p (j f) -> p j f", f=IN_DIM),
                            op=mybir.AluOpType.mult,
                        )
                    # score[e, (j,h)] = reduce_d kq
                    score = grpp.tile([P, J * NUM_HEADS], F32, name="score")
                    nc.vector.tensor_reduce(
                        out=score,
                        in_=kq.rearrange("p (jh d) -> p jh d", d=OUT_DIM),
                        axis=mybir.AxisListType.X,
                        op=mybir.AluOpType.add,
                    )
                    msg = grpp.tile([P, J * IN_DIM], BF16, name="msg")
                    if ACT_REP:
                        score_rep = grpp.tile([P, J * IN_DIM], BF16, name="score_rep")
                        nc.scalar.copy(
                            score_rep.rearrange("p (jh d) -> p jh d", d=OUT_DIM),
                            score.unsqueeze(-1).to_broadcast(
                                [P, J * NUM_HEADS, OUT_DIM]
                            ),
                        )
                        nc.vector.tensor_tensor(
                            out=msg.rearrange("p (j f) -> p j f", f=IN_DIM),
                            in0=v3,
                            in1=score_rep.rearrange("p (j f) -> p j f", f=IN_DIM),
                            op=mybir.AluOpType.mult,
                        )
                    else:
                        for j in range(J):
                            nc.vector.tensor_tensor(
                                out=msg[:, j * IN_DIM : (j + 1) * IN_DIM].rearrange(
                                    "p (h d) -> p h d", d=OUT_DIM
                                ),
                                in0=v3[:, j, :].rearrange("p (h d) -> p h d", d=OUT_DIM),
                                in1=score[:, j * NUM_HEADS : (j + 1) * NUM_HEADS]
                                .unsqueeze(-1)
                                .to_broadcast([P, NUM_HEADS, OUT_DIM]),
                                op=mybir.AluOpType.mult,
                            )
                    # scatter-accumulate into block tiles
                    for j in range(J):
                        sc = sc0 + j
                        b = blk_of[sc]
                        if first_of[sc]:
                            wv_tiles[b] = wvps.tile([P, IN_DIM], F32, name="wv_tile")
                        nc.tensor.matmul(
                            wv_tiles[b],
                            lhsT=onehot[:, j * P : (j + 1) * P],
                            rhs=msg[:, j * IN_DIM : (j + 1) * IN_DIM],
                            start=first_of[sc],
                            stop=last_of[sc],
                        )
                        if last_of[sc]:
                            stage = outp.tile([P, IN_DIM], F32, name="stage")
                            nc.scalar.copy(stage, wv_tiles.pop(b))
                            nc.sync.dma_start(wv_out[b * P : (b + 1) * P, :], stage)
            stack.close()

    nc.finalize()
    return nc


def _make_in_maps(h, h_add, WQ, WK, WV, sched, n_nodes_pad, nodes_core_pad):
    h = np.asarray(h, dtype=np.float32)
    h_add = np.asarray(h_add, dtype=np.float32)
    hT = np.zeros((P, n_nodes_pad), dtype=ml_dtypes.bfloat16)
    hT[:, :N_NODES] = _bf(h.T)
    wkv = np.concatenate(
        [np.asarray(WK, np.float32), np.asarray(WV, np.float32)], axis=1
    )
    wkv = _bf(wkv)
    wq = _bf(np.asarray(WQ, np.float32) / float(N_NODES))
    in_maps = []
    for c in range(N_CORES):
        hqT = np.zeros((P, nodes_core_pad), dtype=ml_dtypes.bfloat16)
        hqT[:, :NODES_PER_CORE] = _bf(
            h_add[c * NODES_PER_CORE : (c + 1) * NODES_PER_CORE].T
        )
        in_maps.append(
            {
                "hT_d": hT,
                "hqT_d": hqT,
                "wkv_d": wkv,
                "wq_d": wq,
                "fused_d": sched["fused"][c],
                "ohT_d": sched["onehotT"][c],
            }
        )
    return in_maps


_TRACE = {"trace": False, "last": None, "tmpdir": None}


def kernel(h, h_add, src, dst, WQ, WK, WV):
    sched = shard_edges(src, dst)
    n_nodes_pad = _ceil_to(N_NODES, P)
    nodes_core_pad = _ceil_to(NODES_PER_CORE, P)

    nc = build_program(
        n_nodes_pad=n_nodes_pad, nodes_core_pad=nodes_core_pad, sched=sched
    )
    in_maps = _make_in_maps(h, h_add, WQ, WK, WV, sched, n_nodes_pad, nodes_core_pad)

    res = bass_utils.run_bass_kernel_spmd(
        nc,
        in_maps,
        core_ids=list(range(N_CORES)),
        trace=_TRACE["trace"],
        tmpdir=_TRACE["tmpdir"],
    )
    _TRACE["last"] = res

    out = np.concatenate(
        [np.asarray(res.results[c]["wv_out"])[:NODES_PER_CORE] for c in range(N_CORES)],
        axis=0,
    )
    return out.reshape(N_NODES, NUM_HEADS, OUT_DIM).astype(np.float32)
